# revision 1
# baseline (speedup 1.0000x reference)
"""Direct 3x3 valid conv (Winograd-equivalent output) on 8 TRN2 cores.

Problem: x [8, 64, 128, 128] f32, filt [64, 64, 3, 3] f32
         -> y [8, 64, 126, 126] f32  (valid correlation, stride 1)

Sharding: 8 cores = 4 sample-pairs x 2 H-halves.
  core c: p = c // 2 (samples 2p, 2p+1), h = c % 2 (output rows h*63 .. h*63+62).
  Each core computes both samples at once: SBUF partitions 0-63 hold sample
  2p's 64 channels, partitions 64-127 hold sample 2p+1's. The per-tap weight
  matrix is block-diagonal [128, 128] so one matmul does both samples.

Compute: implicit GEMM. For each 4-output-row chunk (4*126 = 504 <= 512 PSUM
bank f32 limit), accumulate 9 tap matmuls into PSUM:
  psum[(s,k), pix] += sum_c filt[k, c, dy, dx] * x[s, c, r+dy, w+dx]
then evict PSUM -> SBUF (vector copy) -> DMA to HBM.
"""

import numpy as np

import concourse.bass as bass
import concourse.mybir as mybir
import concourse.tile as tile
from concourse import bacc
from concourse.bass_utils import run_bass_kernel_spmd

N, C, H, W = 8, 64, 128, 128
K = 64
OH = OW = H - 2          # 126
HALF = OH // 2           # 63 output rows per core
IN_ROWS = HALF + 2       # 65 input rows per core
N_CORES = 8
TAPS = [(dy, dx) for dy in range(3) for dx in range(3)]
ROWS_PER_CHUNK = 4       # 4*126 = 504 <= 512 f32 per PSUM bank
N_CHUNKS = (HALF + ROWS_PER_CHUNK - 1) // ROWS_PER_CHUNK  # 16

# float32r = PE fast-fp32 path (1 cyc/row at free>=256 vs 4 for plain fp32).
MM_DT = mybir.dt.float32r

_cache = {}


def _build_nc():
    nc = bacc.Bacc(None)
    xs = nc.dram_tensor("xs", [128, IN_ROWS, W], MM_DT, kind="ExternalInput")
    wt = nc.dram_tensor("wt", [128, 9, 128], MM_DT, kind="ExternalInput")
    out = nc.dram_tensor("out", [128, HALF, OW], mybir.dt.float32, kind="ExternalOutput")

    with tile.TileContext(nc) as tc:
        with (
            tc.tile_pool(name="xpool", bufs=1) as xpool,
            tc.tile_pool(name="wpool", bufs=1) as wpool,
            tc.tile_pool(name="opool", bufs=6) as opool,
            tc.tile_pool(name="psum", bufs=7, space="PSUM") as psum,
        ):
            xs_sb = xpool.tile([128, IN_ROWS, W], MM_DT)
            wt_sb = wpool.tile([128, 9, 128], MM_DT)
            # PE warmup: dummy matmuls on a zeroed scratch tile keep the PE
            # busy through the HAM activity window while the input loads, so
            # real matmuls run at 2.4 GHz from the start.
            warm_sb = wpool.tile([128, 512], mybir.dt.bfloat16, tag="warm_sb")
            nc.gpsimd.memset(warm_sb[:], 0.0)
            warm_ps = psum.tile([128, 504], mybir.dt.float32, tag="warm_ps", bufs=1)
            for _ in range(24):
                nc.tensor.matmul(
                    warm_ps[:], warm_sb[:, 0:128], warm_sb[:, :504],
                    start=True, stop=True,
                )
            # wt on the scalar queue in parallel with band 0 on the sync
            # queue; out-DMAs (also scalar) only start much later. Banded
            # input load in FIFO order gives band 0 priority; deps are
            # byte-range precise so chunk c starts once its bands landed.
            nc.sync.dma_start(wt_sb[:], wt[:])
            bands = [(0, 6)] + [(b, min(b + 8, IN_ROWS)) for b in range(6, IN_ROWS, 8)]
            for b0, b1 in bands:
                nc.sync.dma_start(xs_sb[:, b0:b1, :], xs[:, b0:b1, :])

            # Chunk pairs with interleaved taps: two PSUM banks accumulate in
            # flight so the PE never idles at an accumulation-group boundary.
            for cp in range(0, N_CHUNKS, 2):
                pss = []
                chunk_rows = []
                for ci in (cp, cp + 1):
                    r0 = ci * ROWS_PER_CHUNK
                    rows = min(ROWS_PER_CHUNK, HALF - r0)
                    chunk_rows.append((ci, r0, rows))
                    pss.append(
                        psum.tile(
                            [128, ROWS_PER_CHUNK, OW], mybir.dt.float32,
                            tag="ps", name=f"ps_{ci}",
                        )
                    )
                for ti, (dy, dx) in enumerate(TAPS):
                    for (ci, r0, rows), ps in zip(chunk_rows, pss):
                        rhs = xs_sb[:, r0 + dy : r0 + dy + rows, dx : dx + OW]
                        nc.tensor.matmul(
                            ps[:, :rows, :],
                            wt_sb[:, ti, :],
                            rhs,
                            start=(ti == 0),
                            stop=(ti == len(TAPS) - 1),
                        )
                for (ci, r0, rows), ps in zip(chunk_rows, pss):
                    ob = opool.tile([128, ROWS_PER_CHUNK, OW], mybir.dt.float32, tag="ob")
                    nc.vector.tensor_copy(ob[:, :rows, :], ps[:, :rows, :])
                    nc.scalar.dma_start(out[:, r0 : r0 + rows, :], ob[:, :rows, :])

    nc.finalize()
    return nc


def _shard_inputs(x, filt):
    # Block-diagonal per-tap weights, identical on every core.
    wt = np.zeros((128, 9, 128), dtype=np.float32)
    for ti, (dy, dx) in enumerate(TAPS):
        blk = np.ascontiguousarray(filt[:, :, dy, dx].T)  # [c, k]
        wt[0:64, ti, 0:64] = blk
        wt[64:128, ti, 64:128] = blk

    in_maps = []
    for core in range(N_CORES):
        p, h = core // 2, core % 2
        r0 = h * HALF
        xs = np.empty((128, IN_ROWS, W), dtype=np.float32)
        xs[0:64] = x[2 * p, :, r0 : r0 + IN_ROWS, :]
        xs[64:128] = x[2 * p + 1, :, r0 : r0 + IN_ROWS, :]
        in_maps.append({"xs": xs, "wt": wt})
    return in_maps


def _gather(results):
    y = np.empty((N, K, OH, OW), dtype=np.float32)
    for core in range(N_CORES):
        p, h = core // 2, core % 2
        r0 = h * HALF
        o = results[core]["out"]
        y[2 * p, :, r0 : r0 + HALF, :] = o[0:64]
        y[2 * p + 1, :, r0 : r0 + HALF, :] = o[64:128]
    return y


def kernel(x, filt, **run_kwargs):
    x = np.asarray(x, dtype=np.float32)
    filt = np.asarray(filt, dtype=np.float32)
    if "nc" not in _cache:
        _cache["nc"] = _build_nc()
    in_maps = _shard_inputs(x, filt)
    res = run_bass_kernel_spmd(_cache["nc"], in_maps, list(range(N_CORES)), **run_kwargs)
    _cache["last_results"] = res
    return _gather(res.results)



# revision 2
# speedup vs baseline: 1.1822x; 1.1822x over previous
"""Winograd F(2,3) along H + direct along W, bf16, on 8 TRN2 cores.

Problem: x [8, 64, 128, 128] f32, filt [64, 64, 3, 3] f32
         -> y [8, 64, 126, 126] f32  (valid correlation, stride 1)

Per 2 output rows (one tile-row tr), 1D Winograd along H replaces the
6 row-taps (2 rows x 3 dy) with 4 transformed components
  v0 = x[2tr] - x[2tr+2]    v1 = x[2tr+1] + x[2tr+2]
  v2 = x[2tr+2] - x[2tr+1]  v3 = x[2tr+1] - x[2tr+3]
  m_a = sum_dx U[a,dx] conv_W v_a          (12 matmuls per 4-tile-row chunk
                                            vs 18 direct: 1.5x less PE work)
  y[2tr]   = m0 + m1 + m2
  y[2tr+1] = m1 - m2 - m3
U[a,dx] = sum_dy G[a,dy] filt[:,:,dy,dx] folded on the host; U2, U3 are
negated on the host (m2' = -m2, m3' = -m3) so that both inverse temp sums
become adds and merge into a single strided DVE op:
  [t01 | s23] = [m0 + m1 | m2' + m3']   (one tensor_tensor, stride-2 comps)
  y[2tr]   = t01 - m2'
  y[2tr+1] = m1 + s23

All transforms run on DVE only (concurrent GpSimd tensor ops interfere
with DVE SBUF ports, measured 3x slowdown). bf16 tensor_tensor hits the
2x packed mode (~0.52 ns/elem + ~130 ns overhead), so ops are merged:
v0 & v3 share one subtract op ([2, T, 128] strided), and the forward
transform runs at 8-tile-row granularity (two MM chunks at a time).

Engine split per 4-tile-row chunk (~2.6-2.8 us cadence, pipelined):
  PE     12 matmuls (bf16, free=504, d-major interleaved over 4 banks)
  ACT    one bulk PSUM->SBUF bf16 evict of all 4 components
  DVE    forward transform + 3 inverse ops
  SP     input band DMAs, output DMAs

Sharding: 8 cores = 4 sample-pairs x 2 H-halves, 2 samples stacked on the
128 SBUF partitions with block-diagonal weights. Each core: 32 tile-rows
(64 output rows); the h=1 core overlaps the h=0 core by one tile-row so
both halves run the same SPMD program.
"""

import numpy as np
import ml_dtypes

import concourse.bass as bass
import concourse.mybir as mybir
import concourse.tile as tile
from concourse import bacc
from concourse.bass_utils import run_bass_kernel_spmd

BF16 = ml_dtypes.bfloat16
ADD = mybir.AluOpType.add
SUB = mybir.AluOpType.subtract

N, C, H, W = 8, 64, 128, 128
K = 64
OH = OW = H - 2          # 126
N_CORES = 8
TR = 32                  # tile-rows per core (2 output rows each)
IN_ROWS = 2 * TR + 2     # 66 input rows per core
OUT_ROWS = 2 * TR        # 64 output rows per core
CTR = 4                  # tile-rows per MM chunk
N_CHUNKS = TR // CTR     # 8

MM_DT = mybir.dt.bfloat16
F32 = mybir.dt.float32

_cache = {}


def _build_nc():
    nc = bacc.Bacc(None)
    xs = nc.dram_tensor("xs", [128, IN_ROWS, W], MM_DT, kind="ExternalInput")
    wt = nc.dram_tensor("wt", [128, 12, 128], MM_DT, kind="ExternalInput")
    out = nc.dram_tensor("out", [128, OUT_ROWS, OW], MM_DT, kind="ExternalOutput")

    with tile.TileContext(nc) as tc:
        with (
            tc.tile_pool(name="xpool", bufs=1) as xpool,
            tc.tile_pool(name="wpool", bufs=1) as wpool,
            tc.tile_pool(name="vpool", bufs=3) as vpool,
            tc.tile_pool(name="cpool", bufs=2) as cpool,
            tc.tile_pool(name="tpool", bufs=2) as tpool,
            tc.tile_pool(name="opool", bufs=3) as opool,
            tc.tile_pool(name="psum", bufs=2, space="PSUM") as psum,
        ):
            xs_sb = xpool.tile([128, IN_ROWS, W], MM_DT)
            wt_sb = wpool.tile([128, 12, 128], MM_DT)
            warm_sb = wpool.tile([128, 4, 128], MM_DT, tag="warm_sb")
            nc.vector.memset(warm_sb[:], 0.0)
            warm_ps = psum.tile([128, 4, CTR, 128], F32, tag="M", name="warm_ps")
            for _ in range(8):
                nc.tensor.matmul(
                    warm_ps[:, 0, :, 0:OW], warm_sb[:, 0, :], warm_sb[:, :, 0:OW],
                    start=True, stop=True,
                )
            # weights on the ACT queue in parallel with banded input on SP.
            nc.scalar.dma_start(wt_sb[:], wt[:])
            bands = [(0, 18)] + [(b, min(b + 16, IN_ROWS)) for b in range(18, IN_ROWS, 16)]
            for b0, b1 in bands:
                nc.sync.dma_start(xs_sb[:, b0:b1, :], xs[:, b0:b1, :])

            def fwd(tr0, ntr):
                """Forward transform of tile-rows [tr0, tr0+ntr) -> V tile.
                V comps: 0: v0, 1: v3, 2: v1, 3: v2 (v0/v3 share one op)."""
                r = 2 * tr0
                V = vpool.tile([128, 4, 2 * CTR, W], MM_DT, tag="V", name=f"V_{tr0}")
                # v0 = x[2t] - x[2t+2]; v3 = x[2t+1] - x[2t+3]  (one op):
                # rows r..r+2ntr-1 viewed as (t, pair) -> transposed to (pair, t)
                a0 = xs_sb[:, r : r + 2 * ntr, :].rearrange(
                    "p (t two) w -> p two t w", two=2
                )
                a2 = xs_sb[:, r + 2 : r + 2 + 2 * ntr, :].rearrange(
                    "p (t two) w -> p two t w", two=2
                )
                nc.vector.tensor_tensor(V[:, 0:2, 0:ntr, :], a0, a2, SUB)
                # v1 = x[2t+1] + x[2t+2]
                nc.vector.tensor_tensor(
                    V[:, 2, 0:ntr, :],
                    xs_sb[:, r + 1 : r + 2 * ntr : 2, :],
                    xs_sb[:, r + 2 : r + 1 + 2 * ntr : 2, :],
                    ADD,
                )
                # v2 = x[2t+2] - x[2t+1]
                nc.vector.tensor_tensor(
                    V[:, 3, 0:ntr, :],
                    xs_sb[:, r + 2 : r + 1 + 2 * ntr : 2, :],
                    xs_sb[:, r + 1 : r + 2 * ntr : 2, :],
                    SUB,
                )
                return V

            # M comps: 0: m0, 1: m1, 2: m2' = -m2, 3: m3' = -m3
            # V comp for M comp: m0 <- V0 (v0), m1 <- V2 (v1),
            #                    m2' <- V3 (v2, U negated), m3' <- V1 (v3, U neg)
            M_FROM_V = [0, 2, 3, 1]

            def mms(tr0, ntr, V, voff):
                M = psum.tile([128, 4, CTR, 128], F32, tag="M", name=f"M_{tr0}")
                for d in range(3):
                    for a in range(4):
                        nc.tensor.matmul(
                            M[:, a, 0:ntr, 0:OW],
                            wt_sb[:, a * 3 + d, :],
                            V[:, M_FROM_V[a], voff : voff + ntr, d : d + OW],
                            start=(d == 0),
                            stop=(d == 2),
                        )
                return M

            def inverse(tr0, ntr, M):
                """Evict M (ACT), combine into output rows (DVE), DMA (SP)."""
                cb = cpool.tile([128, 4, CTR, OW], MM_DT, tag="cb", name=f"cb_{tr0}")
                nc.scalar.copy(cb[:, :, 0:ntr, :], M[:, :, 0:ntr, 0:OW])
                # [t01 | s23] = [m0 + m1 | m2' + m3']  (one op, stride-2 comps)
                ts = tpool.tile([128, 2, CTR, OW], MM_DT, tag="ts", name=f"ts_{tr0}")
                ob = opool.tile([128, CTR, 2, OW], MM_DT, tag="ob", name=f"ob_{tr0}")
                nc.vector.tensor_tensor(ts[:, :, 0:ntr, :], cb[:, 0::2, 0:ntr, :], cb[:, 1::2, 0:ntr, :], ADD)
                nc.vector.tensor_tensor(ob[:, 0:ntr, 0, :], ts[:, 0, 0:ntr], cb[:, 2, 0:ntr], SUB)
                nc.vector.tensor_tensor(ob[:, 0:ntr, 1, :], cb[:, 1, 0:ntr], ts[:, 1, 0:ntr], ADD)
                nc.sync.dma_start(
                    out[:, 2 * tr0 : 2 * (tr0 + ntr), :], ob[:, 0:ntr, :, :]
                )

            # Uniform 4-tile-row MM chunks; fwd runs ahead at 8tr granularity.
            chunks = [4] * 8
            fwd_spans = {0: 8, 8: 8, 16: 8, 24: 8}
            pending = None  # (tr0, ntr, M) awaiting inverse
            V = None
            vbase = 0
            tr0 = 0
            for ntr in chunks:
                if tr0 in fwd_spans:
                    V = fwd(tr0, fwd_spans[tr0])
                    vbase = tr0
                M = mms(tr0, ntr, V, tr0 - vbase)
                if pending is not None:
                    inverse(*pending)
                pending = (tr0, ntr, M)
                tr0 += ntr
            inverse(*pending)

    nc.finalize()
    return nc


# 1D Winograd filter transform along dy (reference G)
G = np.array(
    [[1.0, 0.0, 0.0], [0.5, 0.5, 0.5], [0.5, -0.5, 0.5], [0.0, 0.0, 1.0]],
    dtype=np.float32,
)
# host-side sign fold: m2' = -m2, m3' = -m3
SIGN = np.array([1.0, 1.0, -1.0, -1.0], dtype=np.float32)


def _shard_inputs(x, filt):
    # U[a, dx, c, k] = sign[a] * sum_dy G[a, dy] filt[k, c, dy, dx],
    # block-diagonal for the 2 stacked samples. Identical on every core.
    U = np.einsum("a,ad,kcdx->axck", SIGN, G, filt)  # [4, 3, c, k]
    wt = np.zeros((128, 12, 128), dtype=BF16)
    for a in range(4):
        for d in range(3):
            blk = U[a, d].astype(BF16)  # [c, k]
            wt[0:64, a * 3 + d, 0:64] = blk
            wt[64:128, a * 3 + d, 64:128] = blk

    xb = x.astype(BF16)
    in_maps = []
    for core in range(N_CORES):
        p, h = core // 2, core % 2
        r0 = h * (OH - OUT_ROWS)  # 0 or 62
        xcs = np.empty((128, IN_ROWS, W), dtype=BF16)
        xcs[0:64] = xb[2 * p, :, r0 : r0 + IN_ROWS, :]
        xcs[64:128] = xb[2 * p + 1, :, r0 : r0 + IN_ROWS, :]
        in_maps.append({"xs": xcs, "wt": wt})
    return in_maps


def _gather(results):
    y = np.empty((N, K, OH, OW), dtype=np.float32)
    for core in range(N_CORES):
        p, h = core // 2, core % 2
        o = results[core]["out"].astype(np.float32)
        if h == 0:
            y[2 * p, :, 0:OUT_ROWS, :] = o[0:64]
            y[2 * p + 1, :, 0:OUT_ROWS, :] = o[64:128]
        else:
            # local rows 2..63 -> global rows 64..125
            y[2 * p, :, OUT_ROWS:OH, :] = o[0:64, 2:OUT_ROWS]
            y[2 * p + 1, :, OUT_ROWS:OH, :] = o[64:128, 2:OUT_ROWS]
    return y


def kernel(x, filt, **run_kwargs):
    x = np.asarray(x, dtype=np.float32)
    filt = np.asarray(filt, dtype=np.float32)
    if "nc" not in _cache:
        _cache["nc"] = _build_nc()
    in_maps = _shard_inputs(x, filt)
    res = run_bass_kernel_spmd(_cache["nc"], in_maps, list(range(N_CORES)), **run_kwargs)
    _cache["last_results"] = res
    return _gather(res.results)
